# revision 54
# baseline (speedup 1.0000x reference)
"""Multi-head attention layer (L=2048, B=2, D=1024, H=16) on 8 Trainium2 cores.

Sharding: batch*heads across cores — core c handles batch c//4, heads
4*(c%4)..4*(c%4)+4.  Tensor-parallel W_in column slice (per-head) and W_out
row slice; per-core partial outputs are summed on the host (2 groups of 4).

Device program (identical SPMD program, per-core data).  The steady state is
PE-bound (~164us of matmul at f32r rate); everything else is arranged to
keep the tensor engine streaming:

  - inputs ride fp16 (xT (D,L), wqkT, wvT) / f32r (woT) in a few large DMAs
    (the HWDGE issue stage is a single 625ns/DMA resource) ordered so the
    projection waves chase the landings; dummy matmuls warm the PE p-state
    ramp during the initial DMA wait.
  - the softmax exp is split across two engines per head parity: even heads
    exp on ACT (exp(S*32)); odd heads run a custom 8-stage DVE op
    E = ((u+A)^2+B)^32 with u = S/256 (q weights pre-scaled 1/256 on the
    host).  The c0^32 scale cancels in the softmax ratio because each
    head's full rows stay on one engine.  This halves the former ACT
    bottleneck (~144us -> ~2x 100us in parallel).
  - attention blocks are software-pipelined: AV matmuls lag the S matmuls
    by one chunk (two for the 512-wide half-blocks, whose score tiles pack
    both heads into one psum tile) so exp latency is off the critical path.
  - v projection lands token-major with interleaved ones-columns so the AV
    matmul produces z^T and the softmax row sums in one pass; normalization
    is spread over DVE (reciprocal_approx), ACT (psum drain) and GPSIMD
    (multiply), keeping the DVE exp queue short at block boundaries.
  - remaining q/k projection chains sit at block boundaries as PE filler
    sized to the normalization drain; the final head-pair runs as two
    512-wide half-blocks so 12 of 16 output-projection tiles interleave
    into attention and only 4 remain in the serial tail; outputs ship as
    512KB paired DMAs (singles/halves at the very end to shorten the
    drain).
"""

import sys

for _p in ("/opt/trn_rl_repo",):
    if _p not in sys.path:
        sys.path.append(_p)

import numpy as np

L, B, D, H = 2048, 2, 1024, 16
HD = 64
NCORES = 8
HPC = 4              # heads per core
J = HPC * HD         # 256 per-core head-dim slice
KC = D // 128        # 8 contraction chunks
P = 128

_COMPILED = None

# DVE-offloaded exp: half the heads' softmax exponentials run on the Vector
# engine as a custom op E = ((u+A)^2+B)^32 with u = S/256 (q weights are
# pre-scaled 1/256 on the host so u arrives directly from the matmul).  The
# quadratic is a free-scale minimax fit of e^u on |u|<=0.2; the c0^32 global
# factor cancels in the softmax ratio because each head's full row uses one
# engine.  Rel err ~1e-2 at the +-6.4 sigma edges, ~4e-3 end-to-end per head.
_EXP_A = 1.0074990416831697
_EXP_B = 0.9900523184693841


def _register_exp_op():
    import concourse.dve_ops as dve_ops_mod
    from concourse.dve_ops import DveOp
    from concourse.dve_spec import Spec, Src0, C1, C2, sq, lower, _has_src1
    from concourse.dve_uop import DveOpSpec
    from concourse.dve_table_gen import dve_ver_for

    name = "EXP_MQ32_ANT"
    for op in dve_ops_mod.OPS:
        if op.name == name:
            return op

    p = sq(Src0 + C1) + C2
    for _ in range(5):
        p = sq(p)

    def ref(in0, in1, s0, s1, imm2):
        u = np.asarray(in0, np.float32)
        a = (u + np.float32(s1)).astype(np.float32)
        r = (a * a + np.float32(imm2)).astype(np.float32)
        for _ in range(5):
            r = (r * r).astype(np.float32)
        return r

    spec = Spec(body=p, reference=ref)
    row = max(dve_ops_mod._SUB_OPCODE_FOR_NAME.values()) + 1
    assert row < 0x20
    dve_ops_mod._SUB_OPCODE_FOR_NAME[name] = row
    ver = dve_ver_for("TRN2")
    sha = DveOpSpec(
        name=name, opcode=row, uops=lower(spec, ver=ver), rd1_en=_has_src1(spec)
    ).sha(ver)
    op = DveOp(name, spec, subdim=False, uops_sha={ver: sha})
    dve_ops_mod.OPS.append(op)
    dve_ops_mod.CUSTOM_DVE_SPECS[name] = spec
    return op


def _build():
    import concourse.bacc as bacc
    import concourse.mybir as mybir
    import concourse.tile as tile
    from contextlib import ExitStack

    exp_op = _register_exp_op()

    f32 = mybir.dt.float32
    f32r = mybir.dt.float32r
    f16 = mybir.dt.float16
    Exp = mybir.ActivationFunctionType.Exp
    Mult = mybir.AluOpType.mult

    nc = bacc.Bacc("TRN2", target_bir_lowering=False, debug=False)

    # x and the in-projection weights ride as fp16 (10-bit mantissa, on
    # par with f32r) to halve the prologue DMA; attention stays f32r
    xT_d = nc.dram_tensor("xT", (D, L), f16, kind="ExternalInput")
    wqk_d = nc.dram_tensor("wqkT", (D, 2 * J), f16, kind="ExternalInput")
    wv_d = nc.dram_tensor("wvT", (D, J), f16, kind="ExternalInput")
    wo_d = nc.dram_tensor("woT", (J, D), f32r, kind="ExternalInput")
    out_d = nc.dram_tensor("out_p", (L, D), f16, kind="ExternalOutput")

    with tile.TileContext(nc) as tc, ExitStack() as ctx:
        pers = ctx.enter_context(tc.tile_pool(name="pers", bufs=1))
        psum = ctx.enter_context(tc.tile_pool(name="psum", bufs=2, space="PSUM"))
        # z-tag tiles are [P,512] halves; 4 slots cover 2 heads x 2 halves
        att = ctx.enter_context(tc.tile_pool(name="att", bufs=3))

        qk_sb = pers.tile([P, 4, L], f32r)          # chunks 0,1: q^T; 2,3: k^T
        v_sb = pers.tile([P, 16, HPC, P], f32r)     # ones cols 0:64, v 64:128

        proj_a = ctx.enter_context(tc.tile_pool(name="proj_a", bufs=1))
        xT_sb = proj_a.tile([P, KC, L], f16)
        wqk_sb = proj_a.tile([P, KC, 2 * J], f16)

        out_ap = out_d.ap().rearrange("(t p) o -> p t o", p=P)

        def qk_proj(jc, tag, fine=False, q4s=(0, 1, 2, 3), copy_eng=None):
            if fine:
                # 512-wide psum chunks: shorter slot holds when slotted
                # between attention blocks on the shared z tag; copy_eng
                # picks which engine drains psum (boundary "gating" chunks
                # ride ACT so they don't queue behind normalization on DVE)
                for q4 in q4s:
                    m0 = q4 * 512
                    pt = psum.tile([P, 512], f32, tag=tag,
                                   bufs=(4 if tag == "z" else 2),
                                   name=f"qkp_{jc}_{q4}")
                    for kc in range(KC):
                        nc.tensor.matmul(
                            pt[:],
                            wqk_sb[:, kc, jc * P:(jc + 1) * P],
                            xT_sb[:, kc, m0:m0 + 512],
                            start=(kc == 0), stop=(kc == KC - 1),
                        )
                    if copy_eng == "scalar":
                        nc.scalar.copy(qk_sb[:, jc, m0:m0 + 512], pt[:])
                    else:
                        nc.vector.tensor_copy(
                            qk_sb[:, jc, m0:m0 + 512], pt[:]
                        )
                return
            for mh in range(2):
                pt = psum.tile([P, 1024], f32, tag=tag, name=f"qkp_{jc}_{mh}")
                for q2 in range(2):
                    m0 = mh * 1024 + q2 * 512
                    for kc in range(KC):
                        nc.tensor.matmul(
                            pt[:, q2 * 512:(q2 + 1) * 512],
                            wqk_sb[:, kc, jc * P:(jc + 1) * P],
                            xT_sb[:, kc, m0:m0 + 512],
                            start=(kc == 0), stop=(kc == KC - 1),
                        )
                nc.vector.tensor_copy(
                    qk_sb[:, jc, mh * 1024:(mh + 1) * 1024], pt[:]
                )

        # output tiles are buffered in pairs so each DMA moves 512KB — the
        # HWDGE issue stage (625ns, globally serialized) is the tail pacer
        out_pair = {}

        def out_proj(t, zn_sb, wo_sb, tag, use_act=False):
            pos = [psum.tile([P, 512], f32, tag="z", bufs=4,
                              name=f"po_{t}_{oc}")
                   for oc in range(2)]
            for oc in range(2):
                for dc in range(2):
                    nc.tensor.matmul(
                        pos[oc][:],
                        zn_sb[:, dc, t * P:(t + 1) * P],
                        wo_sb[:, dc, oc * 512:(oc + 1) * 512],
                        start=(dc == 0), stop=(dc == 1),
                    )
            if t % 2 == 0:
                out_pair[0] = att.tile([P, 2, 1024], f16, tag="o", bufs=2,
                                       name=f"op_{t}")
            pair = out_pair[0]
            nc.scalar.copy(pair[:, t % 2, 0:512], pos[0][:])
            if use_act:
                nc.scalar.copy(pair[:, t % 2, 512:1024], pos[1][:])
            else:
                nc.vector.tensor_copy(pair[:, t % 2, 512:1024], pos[1][:])
            if t == 15:
                # final halves issue from the (idle) ACT/DVE queues so the
                # SP sequencer's ~700ns-per-issue serialization of the t12-14
                # DMAs doesn't delay the very last transfer
                nc.scalar.dma_start(out_ap[:, 15, 0:512], pair[:, 1, 0:512])
                nc.sync.dma_start(out_ap[:, 15, 512:1024],
                                  pair[:, 1, 512:1024])
            elif t == 14:
                nc.scalar.dma_start(out_ap[:, 14, :], pair[:, 0])  # ACT queue
            elif t >= 12:
                nc.sync.dma_start(out_ap[:, t, :], pair[:, t % 2])
            elif t % 2 == 1:
                nc.sync.dma_start(out_ap[:, t - 1:t + 1, :], pair[:])

        with tc.tile_pool(name="proj_b", bufs=1) as proj_b:
            wv_sb = proj_b.tile([P, KC, J], f16)

            xT_ap = xT_d.ap().rearrange("(kc p) m -> p kc m", p=P)
            wqk_ap = wqk_d.ap().rearrange("(kc p) j -> p kc j", p=P)
            wv_ap = wv_d.ap().rearrange("(kc p) j -> p kc j", p=P)
            # few, large DMAs (the HWDGE issue stage is a single 625ns/DMA
            # resource), ordered so the projection waves below chase the
            # landings: q-weights for the first chain, then xT token-column
            # groups in consumption order
            nc.sync.dma_start(wqk_sb[:, :, 0:128], wqk_ap[:, :, 0:128])
            nc.sync.dma_start(xT_sb[:, 0:4, 0:512], xT_ap[:, 0:4, 0:512])
            nc.sync.dma_start(xT_sb[:, 4:8, 0:512], xT_ap[:, 4:8, 0:512])
            nc.sync.dma_start(wv_sb[:, :, :], wv_ap[:, :, :])
            nc.sync.dma_start(wqk_sb[:, :, 128:512], wqk_ap[:, :, 128:512])
            for cg in range(1, 4):
                nc.sync.dma_start(
                    xT_sb[:, :, cg * 512:(cg + 1) * 512],
                    xT_ap[:, :, cg * 512:(cg + 1) * 512],
                )

            # PE p-state warmup: the tensor engine needs ~3us of continuous
            # execution to reach full clock, so burn the initial DMA wait on
            # dummy matmuls and the projection chains start at full speed
            wz = proj_b.tile([P, 128], f32)
            nc.vector.memset(wz[:], 0.0)
            warm = wz[:].bitcast(f32r)
            for i in range(20):
                wp = psum.tile([P, 128], f32, tag="S", name=f"warm_{i}")
                nc.tensor.matmul(wp[:], warm[:], warm[:],
                                 start=True, stop=True)

            # ones columns at 0:64 for every head — keeps the softmax sums on
            # psum partitions 0-63 where the custom-DVE reciprocal works (it
            # silently corrupts at base partition 64).  memset on an f32r
            # tile fails the ISA check, so round through a f32 scratch tile.
            ones_sc = proj_b.tile([P, 64], f32)
            nc.vector.memset(ones_sc[:], 1.0)
            for h in range(HPC):
                nc.vector.tensor_copy(
                    v_sb[:, :, h, 0:64],
                    ones_sc[:, None, :].to_broadcast((P, 16, 64)),
                )

            # block (0,0) reads q-pair-0 for l 0:1024 but k-pair-0 for the
            # FULL key range; everything else is produced at the block
            # boundary just before its consumer.  Work is emitted in token-
            # column waves matching the xT DMA landing order; v_sb copies ride
            # the otherwise-idle scalar engine
            def v_proj(mg):
                pts = [psum.tile([P, 512], f32, tag="z", bufs=4,
                                 name=f"vp_{mg}_{i}")
                       for i in range(2)]
                for sub in range(4):
                    mc = mg * 4 + sub
                    pt = pts[sub // 2][:, (sub % 2) * 256:(sub % 2 + 1) * 256]
                    for kc in range(KC):
                        nc.tensor.matmul(
                            pt,
                            xT_sb[:, kc, mc * P:(mc + 1) * P],
                            wv_sb[:, kc, :],
                            start=(kc == 0), stop=(kc == KC - 1),
                        )
                for sub in range(4):
                    mc = mg * 4 + sub
                    dst = v_sb[:, mc, :, 64:128]
                    srcp = pts[sub // 2][:, (sub % 2) * 256:(sub % 2 + 1) * 256
                                         ].rearrange("p (h e) -> p h e", e=64)
                    if mg == 3:
                        nc.vector.tensor_copy(dst, srcp)
                    else:
                        nc.scalar.copy(dst, srcp)

            qk_proj(0, "S", fine=True, q4s=(0,))
            v_proj(0)
            qk_proj(2, "S", fine=True, q4s=(0,))
            qk_proj(0, "S", fine=True, q4s=(1,))
            qk_proj(2, "S", fine=True, q4s=(1,))
            v_proj(1)
            qk_proj(2, "S", fine=True, q4s=(2,))
            v_proj(2)
            qk_proj(2, "S", fine=True, q4s=(3,))
            v_proj(3)

        post = ctx.enter_context(tc.tile_pool(name="post", bufs=1))
        zn_sb = post.tile([P, 2, L], f32r)          # normalized z^T
        wo_sb = post.tile([P, 2, D], f32r)
        wo_ap = wo_d.ap().rearrange("(dc p) o -> p dc o", p=P)
        nc.sync.dma_start(wo_sb[:, :, :], wo_ap[:, :, :])

        # attention, head pairs (even head on psum out partitions via v cols;
        # q/k chunks pair heads on partition halves for PE row-group overlap)
        def attn_block(hp, l0, w, interleave=None, tail=False):
            pair = (2 * hp, 2 * hp + 1)
            nq = w // 512
            zts = {
                (h, q2): psum.tile([P, 512], f32, tag="z", bufs=4,
                                   name=f"z_{h}_{q2}_{hp}_{l0}")
                for h in pair for q2 in range(nq)
            }

            def do_av(mc, h, E):
                for q2 in range(nq):
                    nc.tensor.matmul(
                        zts[(h, q2)][:],
                        v_sb[:, mc, h, :],
                        E[:, q2 * 512:(q2 + 1) * 512],
                        start=(mc == 0), stop=(mc == 15),
                    )

            # software-pipelined: AV of an older chunk issues after the S
            # matmuls of chunk mc, so each exp (ACT for even heads, DVE
            # custom op for odd) gets ~1.7us of slack off the critical path.
            # Full blocks lag 1 chunk; half-width blocks lag 2 (iterations
            # are half as long) and pack both heads' scores into one psum
            # tile so two slots cover the deeper pipeline.
            lag = 3 - nq
            pend = []
            for mc in range(16):
                curr = []
                S2 = (psum.tile([P, 1024], f32, tag="S", name=f"S2_{mc}")
                      if nq == 1 else None)
                for hi, h in enumerate(pair):
                    r0 = (h % 2) * 64
                    if nq == 1:
                        S = S2[:, hi * 512:(hi + 1) * 512]
                    else:
                        S_t = psum.tile([P, w], f32, tag="S", name=f"S_{mc}_{h}")
                        S = S_t[:]
                    for q2 in range(nq):
                        nc.tensor.matmul(
                            S[:, q2 * 512:(q2 + 1) * 512],
                            qk_sb[r0:r0 + 64, 2 + h // 2, mc * P:(mc + 1) * P],
                            qk_sb[r0:r0 + 64, h // 2,
                                  l0 + q2 * 512: l0 + (q2 + 1) * 512],
                            start=True, stop=True,
                        )
                    E = att.tile([P, w], f32r, tag="E", bufs=14)
                    # engine choice is strictly per-head: the DVE exp carries
                    # a c0^32 global factor that only cancels in the softmax
                    # ratio if every chunk of a row uses the same engine
                    if h % 2 == 0:
                        nc.scalar.activation(E[:], S[:], Exp, scale=32.0)
                    else:
                        nc.vector._custom_dve(
                            exp_op, out=E[:], in0=S[:],
                            s0=0.0, s1=_EXP_A, imm2=_EXP_B,
                        )
                    curr.append((mc, h, E))
                pend.append(curr)
                if len(pend) > lag:
                    for args in pend.pop(0):
                        do_av(*args)
                if interleave is not None and mc in interleave:
                    interleave[mc]()
            if tail:
                # final block: drain AVs head-major and normalize each head
                # entirely on the DVE the moment its accumulator stops, so
                # the tail out_proj unblocks as early as possible
                for hi, h in enumerate(pair):
                    for chunk in pend:
                        do_av(*chunk[hi])
                    zt = zts[(h, 0)]
                    rz = (h % 2) * 64
                    rsb = att.tile([P, 512], f32, tag="r", bufs=2,
                                   name=f"rt_{h}")
                    nc.vector.reciprocal_approx_fast(out=rsb[0:64, :],
                                                     in_=zt[0:64, :])
                    nc.vector.tensor_tensor(
                        zn_sb[rz:rz + 64, hp, l0:l0 + 512],
                        zt[64:128, :], rsb[0:64, :], Mult,
                    )
                return
            for chunk in pend:
                for args in chunk:
                    do_av(*args)
            # normalization split across three engines so the DVE (which also
            # carries half the exps) frees the z psum slots quickly: DVE only
            # computes reciprocals, ACT drains z out of psum, and the final
            # multiply runs on the otherwise-idle GPSIMD from SBUF.  qh-major
            # so the first token half unblocks consumers sooner
            for qh in range(nq):
                for h in pair:
                    zt = zts[(h, qh)]
                    rz = (h % 2) * 64
                    rsb = att.tile([P, 512], f32, tag="r", bufs=2)
                    nc.vector.reciprocal_approx_fast(out=rsb[0:64, :],
                                                     in_=zt[0:64, :])
                    if w == 512:
                        # half-block: all-DVE norm keeps the z psum release
                        # off the busy ACT queue (shorter b4 boundary)
                        nc.vector.tensor_tensor(
                            zn_sb[rz:rz + 64, hp,
                                  l0 + qh * 512:l0 + (qh + 1) * 512],
                            zt[64:128, :], rsb[0:64, :], Mult,
                        )
                    else:
                        zcp = att.tile([P, 512], f32, tag="zc", bufs=2)
                        nc.scalar.copy(zcp[0:64, :], zt[64:128, :])
                        nc.gpsimd.tensor_tensor(
                            zn_sb[rz:rz + 64, hp,
                                  l0 + qh * 512:l0 + (qh + 1) * 512],
                            zcp[0:64, :], rsb[0:64, :], Mult,
                        )

        # chunks the NEXT block reads go on the S tag (granted immediately
        # at the boundary); later-consumer prefetch rides the z tag and
        # overlaps the next block under the E-buffer slack.  (Producing the
        # gating chunks inside the previous block via S-tag inserts was
        # tried and measured worse — any theft from the S slot stream
        # stalls the exp pipeline more than the boundary gate costs.)
        def ilv(t, use_act=True):
            return lambda: out_proj(t, zn_sb, wo_sb, "S", use_act=use_act)

        # remaining projection chains are spread over the block boundaries
        # as PE filler sized to the DVE's normalization backlog; each chain
        # lands just before its consuming block (gating) or one block early
        # (prefetch)
        attn_block(0, 0, 1024)
        qk_proj(0, "S", fine=True, q4s=(2, 3), copy_eng="scalar")
        qk_proj(3, "z", fine=True, q4s=(0, 1, 2, 3))
        attn_block(0, 1024, 1024)
        qk_proj(1, "S", fine=True, q4s=(0, 1), copy_eng="scalar")
        qk_proj(1, "z", fine=True, q4s=(2, 3))
        attn_block(1, 0, 1024)
        # the last head-pair's q range runs as two 512-wide half-blocks so
        # out_proj tiles 8-11 interleave into the second half; only 12-15
        # remain for the serial tail
        attn_block(1, 1024, 512, interleave={
            mc: ilv((mc - 5) // 2, False) for mc in range(5, 16, 2)
        })
        attn_block(1, 1536, 512, interleave={
            3: ilv(6, False), 5: ilv(7, False), 7: ilv(8, False),
            9: ilv(9, False), 11: ilv(10, False), 13: ilv(11, False),
        }, tail=True)
        for t in range(12, 16):
            out_proj(t, zn_sb, wo_sb, "S")

    nc.compile()
    return nc


def _get_compiled():
    global _COMPILED
    if _COMPILED is None:
        _COMPILED = _build()
    return _COMPILED


def _shard_inputs(x, W_in, W_out):
    in_maps = []
    xTs = [x[:, b, :].T.astype(np.float16) for b in range(B)]
    for c in range(NCORES):
        b = c // 4
        lo = (c % 4) * J
        Wq = W_in[lo:lo + J]
        Wk = W_in[D + lo:D + lo + J]
        Wv = W_in[2 * D + lo:2 * D + lo + J]
        in_maps.append({
            "xT": xTs[b],
            # q weights carry the 1/256 pre-scale so scores arrive as S/256:
            # the ACT exp uses scale=32 (256/8) and the DVE exp op consumes
            # S/256 directly
            "wqkT": np.concatenate([Wq / 256.0, Wk], 0).T.astype(np.float16),
            "wvT": Wv.T.astype(np.float16),
            "woT": np.ascontiguousarray(W_out[:, lo:lo + J].T),
        })
    return in_maps


def _reference_numpy(q, mask, W_in, b_in, W_out, b_out, num_heads):
    l, b, d = q.shape
    hd = d // num_heads
    qkv = q.reshape(l * b, d) @ W_in.T + b_in
    qkv = qkv.reshape(l, b, 3 * d)
    qh, kh, vh = np.split(qkv, 3, axis=-1)

    def to_heads(t):
        return t.reshape(l, b * num_heads, hd).transpose(1, 0, 2)

    qh, kh, vh = to_heads(qh), to_heads(kh), to_heads(vh)
    qh = qh / np.sqrt(np.float32(hd))
    scores = np.einsum("nld,nmd->nlm", qh, kh) + mask
    scores -= scores.max(axis=-1, keepdims=True)
    e = np.exp(scores)
    attn = e / e.sum(axis=-1, keepdims=True)
    z = np.einsum("nlm,nmd->nld", attn, vh)
    z = z.transpose(1, 0, 2).reshape(l * b, d)
    z = z @ W_out.T + b_out
    return z.reshape(l, b, d).astype(np.float32)


def kernel(q, k, v, mask, W_in, b_in, W_out, b_out, num_heads):
    num_heads = int(num_heads)
    q = np.asarray(q, dtype=np.float32)
    W_in = np.asarray(W_in, dtype=np.float32)
    W_out = np.asarray(W_out, dtype=np.float32)
    b_in = np.asarray(b_in, dtype=np.float32)
    b_out = np.asarray(b_out, dtype=np.float32)
    mask = np.asarray(mask, dtype=np.float32)

    if (
        num_heads != H
        or q.shape != (L, B, D)
        or W_in.shape != (3 * D, D)
        or W_out.shape != (D, D)
        or np.any(mask)
        or np.any(b_in)
    ):
        return _reference_numpy(q, mask, W_in, b_in, W_out, b_out, num_heads)

    from concourse import bass_utils

    nc = _get_compiled()
    in_maps = _shard_inputs(q, W_in, W_out)
    res = bass_utils.run_bass_kernel_spmd(
        nc, in_maps, core_ids=list(range(NCORES))
    )

    out = np.zeros((L, B, D), dtype=np.float32)
    for c in range(NCORES):
        out[:, c // 4, :] += res.results[c]["out_p"].astype(np.float32)
    out += b_out
    return out



# revision 55
# speedup vs baseline: 1.0055x; 1.0055x over previous
"""Multi-head attention layer (L=2048, B=2, D=1024, H=16) on 8 Trainium2 cores.

Sharding: batch*heads across cores — core c handles batch c//4, heads
4*(c%4)..4*(c%4)+4.  Tensor-parallel W_in column slice (per-head) and W_out
row slice; per-core partial outputs are summed on the host (2 groups of 4).

Device program (identical SPMD program, per-core data).  The steady state is
PE-bound (~164us of matmul at f32r rate); everything else is arranged to
keep the tensor engine streaming:

  - inputs ride fp16 (xT (D,L), wqkT, wvT) / f32r (woT) in a few large DMAs
    (the HWDGE issue stage is a single 625ns/DMA resource) ordered so the
    projection waves chase the landings; dummy matmuls warm the PE p-state
    ramp during the initial DMA wait.
  - the softmax exp is split across two engines per head parity: even heads
    exp on ACT (exp(S*32)); odd heads run a custom 8-stage DVE op
    E = ((u+A)^2+B)^32 with u = S/256 (q weights pre-scaled 1/256 on the
    host).  The c0^32 scale cancels in the softmax ratio because each
    head's full rows stay on one engine.  This halves the former ACT
    bottleneck (~144us -> ~2x 100us in parallel).
  - attention blocks are software-pipelined: AV matmuls lag the S matmuls
    by one chunk (two for the 512-wide half-blocks, whose score tiles pack
    both heads into one psum tile) so exp latency is off the critical path.
  - v projection lands token-major with interleaved ones-columns so the AV
    matmul produces z^T and the softmax row sums in one pass; normalization
    is spread over DVE (reciprocal_approx), ACT (psum drain) and GPSIMD
    (multiply), keeping the DVE exp queue short at block boundaries.
  - remaining q/k projection chains sit at block boundaries as PE filler
    sized to the normalization drain; the final head-pair runs as two
    512-wide half-blocks so 12 of 16 output-projection tiles interleave
    into attention and only 4 remain in the serial tail; outputs ship as
    512KB paired DMAs (singles/halves at the very end to shorten the
    drain).
"""

import sys

for _p in ("/opt/trn_rl_repo",):
    if _p not in sys.path:
        sys.path.append(_p)

import numpy as np

L, B, D, H = 2048, 2, 1024, 16
HD = 64
NCORES = 8
HPC = 4              # heads per core
J = HPC * HD         # 256 per-core head-dim slice
KC = D // 128        # 8 contraction chunks
P = 128

_COMPILED = None

# DVE-offloaded exp: half the heads' softmax exponentials run on the Vector
# engine as a custom op E = ((u+A)^2+B)^32 with u = S/256 (q weights are
# pre-scaled 1/256 on the host so u arrives directly from the matmul).  The
# quadratic is a free-scale minimax fit of e^u on |u|<=0.2; the c0^32 global
# factor cancels in the softmax ratio because each head's full row uses one
# engine.  Rel err ~1e-2 at the +-6.4 sigma edges, ~4e-3 end-to-end per head.
_EXP_A = 1.0074990416831697
_EXP_B = 0.9900523184693841


def _register_exp_op():
    import concourse.dve_ops as dve_ops_mod
    from concourse.dve_ops import DveOp
    from concourse.dve_spec import Spec, Src0, C1, C2, sq, lower, _has_src1
    from concourse.dve_uop import DveOpSpec
    from concourse.dve_table_gen import dve_ver_for

    name = "EXP_MQ32_ANT"
    for op in dve_ops_mod.OPS:
        if op.name == name:
            return op

    p = sq(Src0 + C1) + C2
    for _ in range(5):
        p = sq(p)

    def ref(in0, in1, s0, s1, imm2):
        u = np.asarray(in0, np.float32)
        a = (u + np.float32(s1)).astype(np.float32)
        r = (a * a + np.float32(imm2)).astype(np.float32)
        for _ in range(5):
            r = (r * r).astype(np.float32)
        return r

    spec = Spec(body=p, reference=ref)
    row = max(dve_ops_mod._SUB_OPCODE_FOR_NAME.values()) + 1
    assert row < 0x20
    dve_ops_mod._SUB_OPCODE_FOR_NAME[name] = row
    ver = dve_ver_for("TRN2")
    sha = DveOpSpec(
        name=name, opcode=row, uops=lower(spec, ver=ver), rd1_en=_has_src1(spec)
    ).sha(ver)
    op = DveOp(name, spec, subdim=False, uops_sha={ver: sha})
    dve_ops_mod.OPS.append(op)
    dve_ops_mod.CUSTOM_DVE_SPECS[name] = spec
    return op


def _build():
    import concourse.bacc as bacc
    import concourse.mybir as mybir
    import concourse.tile as tile
    from contextlib import ExitStack

    exp_op = _register_exp_op()

    f32 = mybir.dt.float32
    f32r = mybir.dt.float32r
    f16 = mybir.dt.float16
    Exp = mybir.ActivationFunctionType.Exp
    Mult = mybir.AluOpType.mult

    nc = bacc.Bacc("TRN2", target_bir_lowering=False, debug=False)

    # x and the in-projection weights ride as fp16 (10-bit mantissa, on
    # par with f32r) to halve the prologue DMA; attention stays f32r
    xT_d = nc.dram_tensor("xT", (D, L), f16, kind="ExternalInput")
    wqk_d = nc.dram_tensor("wqkT", (D, 2 * J), f16, kind="ExternalInput")
    wv_d = nc.dram_tensor("wvT", (D, J), f16, kind="ExternalInput")
    wo_d = nc.dram_tensor("woT", (J, D), f32r, kind="ExternalInput")
    out_d = nc.dram_tensor("out_p", (L, D), f16, kind="ExternalOutput")

    with tile.TileContext(nc) as tc, ExitStack() as ctx:
        pers = ctx.enter_context(tc.tile_pool(name="pers", bufs=1))
        psum = ctx.enter_context(tc.tile_pool(name="psum", bufs=2, space="PSUM"))
        # z-tag tiles are [P,512] halves; 4 slots cover 2 heads x 2 halves
        att = ctx.enter_context(tc.tile_pool(name="att", bufs=3))

        qk_sb = pers.tile([P, 4, L], f32r)          # chunks 0,1: q^T; 2,3: k^T
        v_sb = pers.tile([P, 16, HPC, P], f32r)     # ones cols 0:64, v 64:128

        proj_a = ctx.enter_context(tc.tile_pool(name="proj_a", bufs=1))
        xT_sb = proj_a.tile([P, KC, L], f16)
        wqk_sb = proj_a.tile([P, KC, 2 * J], f16)

        out_ap = out_d.ap().rearrange("(t p) o -> p t o", p=P)

        def qk_proj(jc, tag, fine=False, q4s=(0, 1, 2, 3), copy_eng=None):
            if fine:
                # 512-wide psum chunks: shorter slot holds when slotted
                # between attention blocks on the shared z tag; copy_eng
                # picks which engine drains psum (boundary "gating" chunks
                # ride ACT so they don't queue behind normalization on DVE)
                for q4 in q4s:
                    m0 = q4 * 512
                    pt = psum.tile([P, 512], f32, tag=tag,
                                   bufs=(4 if tag == "z" else 2),
                                   name=f"qkp_{jc}_{q4}")
                    for kc in range(KC):
                        nc.tensor.matmul(
                            pt[:],
                            wqk_sb[:, kc, jc * P:(jc + 1) * P],
                            xT_sb[:, kc, m0:m0 + 512],
                            start=(kc == 0), stop=(kc == KC - 1),
                        )
                    if copy_eng == "scalar":
                        nc.scalar.copy(qk_sb[:, jc, m0:m0 + 512], pt[:])
                    else:
                        nc.vector.tensor_copy(
                            qk_sb[:, jc, m0:m0 + 512], pt[:]
                        )
                return
            for mh in range(2):
                pt = psum.tile([P, 1024], f32, tag=tag, name=f"qkp_{jc}_{mh}")
                for q2 in range(2):
                    m0 = mh * 1024 + q2 * 512
                    for kc in range(KC):
                        nc.tensor.matmul(
                            pt[:, q2 * 512:(q2 + 1) * 512],
                            wqk_sb[:, kc, jc * P:(jc + 1) * P],
                            xT_sb[:, kc, m0:m0 + 512],
                            start=(kc == 0), stop=(kc == KC - 1),
                        )
                nc.vector.tensor_copy(
                    qk_sb[:, jc, mh * 1024:(mh + 1) * 1024], pt[:]
                )

        # output tiles are buffered in pairs so each DMA moves 512KB — the
        # HWDGE issue stage (625ns, globally serialized) is the tail pacer
        out_pair = {}

        def out_proj(t, zn_sb, wo_sb, tag, use_act=False):
            pos = [psum.tile([P, 512], f32, tag="z", bufs=4,
                              name=f"po_{t}_{oc}")
                   for oc in range(2)]
            for oc in range(2):
                for dc in range(2):
                    nc.tensor.matmul(
                        pos[oc][:],
                        zn_sb[:, dc, t * P:(t + 1) * P],
                        wo_sb[:, dc, oc * 512:(oc + 1) * 512],
                        start=(dc == 0), stop=(dc == 1),
                    )
            if t % 2 == 0:
                out_pair[0] = att.tile([P, 2, 1024], f16, tag="o", bufs=2,
                                       name=f"op_{t}")
            pair = out_pair[0]
            nc.scalar.copy(pair[:, t % 2, 0:512], pos[0][:])
            if use_act:
                nc.scalar.copy(pair[:, t % 2, 512:1024], pos[1][:])
            else:
                nc.vector.tensor_copy(pair[:, t % 2, 512:1024], pos[1][:])
            if t == 15:
                # final halves issue from the (idle) ACT/DVE queues so the
                # SP sequencer's ~700ns-per-issue serialization of the t12-14
                # DMAs doesn't delay the very last transfer
                nc.scalar.dma_start(out_ap[:, 15, 0:512], pair[:, 1, 0:512])
                nc.sync.dma_start(out_ap[:, 15, 512:1024],
                                  pair[:, 1, 512:1024])
            elif t == 14:
                nc.scalar.dma_start(out_ap[:, 14, :], pair[:, 0])  # ACT queue
            elif t >= 12:
                nc.sync.dma_start(out_ap[:, t, :], pair[:, t % 2])
            elif t % 2 == 1:
                nc.sync.dma_start(out_ap[:, t - 1:t + 1, :], pair[:])

        with tc.tile_pool(name="proj_b", bufs=1) as proj_b:
            wv_sb = proj_b.tile([P, KC, J], f16)

            xT_ap = xT_d.ap().rearrange("(kc p) m -> p kc m", p=P)
            wqk_ap = wqk_d.ap().rearrange("(kc p) j -> p kc j", p=P)
            wv_ap = wv_d.ap().rearrange("(kc p) j -> p kc j", p=P)
            # few, large DMAs (the HWDGE issue stage is a single 625ns/DMA
            # resource), ordered so the projection waves below chase the
            # landings: q-weights for the first chain, then xT token-column
            # groups in consumption order
            nc.sync.dma_start(wqk_sb[:, :, 0:128], wqk_ap[:, :, 0:128])
            nc.sync.dma_start(xT_sb[:, 0:4, 0:512], xT_ap[:, 0:4, 0:512])
            nc.sync.dma_start(xT_sb[:, 4:8, 0:512], xT_ap[:, 4:8, 0:512])
            nc.sync.dma_start(wv_sb[:, :, :], wv_ap[:, :, :])
            nc.sync.dma_start(wqk_sb[:, :, 128:512], wqk_ap[:, :, 128:512])
            for cg in range(1, 4):
                nc.sync.dma_start(
                    xT_sb[:, :, cg * 512:(cg + 1) * 512],
                    xT_ap[:, :, cg * 512:(cg + 1) * 512],
                )

            # PE p-state warmup: the tensor engine needs ~3us of continuous
            # execution to reach full clock, so burn the initial DMA wait on
            # dummy matmuls and the projection chains start at full speed
            wz = proj_b.tile([P, 128], f32)
            nc.vector.memset(wz[:], 0.0)
            warm = wz[:].bitcast(f32r)
            for i in range(20):
                wp = psum.tile([P, 128], f32, tag="S", name=f"warm_{i}")
                nc.tensor.matmul(wp[:], warm[:], warm[:],
                                 start=True, stop=True)

            # ones columns at 0:64 for every head — keeps the softmax sums on
            # psum partitions 0-63 where the custom-DVE reciprocal works (it
            # silently corrupts at base partition 64).  memset on an f32r
            # tile fails the ISA check, so round through a f32 scratch tile.
            ones_sc = proj_b.tile([P, 64], f32)
            nc.vector.memset(ones_sc[:], 1.0)
            for h in range(HPC):
                nc.vector.tensor_copy(
                    v_sb[:, :, h, 0:64],
                    ones_sc[:, None, :].to_broadcast((P, 16, 64)),
                )

            # block (0,0) reads q-pair-0 for l 0:1024 but k-pair-0 for the
            # FULL key range; everything else is produced at the block
            # boundary just before its consumer.  Work is emitted in token-
            # column waves matching the xT DMA landing order; v_sb copies ride
            # the otherwise-idle scalar engine
            def v_proj(mg):
                pts = [psum.tile([P, 512], f32, tag="z", bufs=4,
                                 name=f"vp_{mg}_{i}")
                       for i in range(2)]
                for sub in range(4):
                    mc = mg * 4 + sub
                    pt = pts[sub // 2][:, (sub % 2) * 256:(sub % 2 + 1) * 256]
                    for kc in range(KC):
                        nc.tensor.matmul(
                            pt,
                            xT_sb[:, kc, mc * P:(mc + 1) * P],
                            wv_sb[:, kc, :],
                            start=(kc == 0), stop=(kc == KC - 1),
                        )
                for sub in range(4):
                    mc = mg * 4 + sub
                    dst = v_sb[:, mc, :, 64:128]
                    srcp = pts[sub // 2][:, (sub % 2) * 256:(sub % 2 + 1) * 256
                                         ].rearrange("p (h e) -> p h e", e=64)
                    if mg == 3:
                        nc.vector.tensor_copy(dst, srcp)
                    else:
                        nc.scalar.copy(dst, srcp)

            qk_proj(0, "S", fine=True, q4s=(0,))
            v_proj(0)
            qk_proj(2, "S", fine=True, q4s=(0,))
            qk_proj(0, "S", fine=True, q4s=(1,))
            qk_proj(2, "S", fine=True, q4s=(1,))
            v_proj(1)
            qk_proj(2, "S", fine=True, q4s=(2,))
            v_proj(2)
            qk_proj(2, "S", fine=True, q4s=(3,))
            v_proj(3)

        post = ctx.enter_context(tc.tile_pool(name="post", bufs=1))
        zn_sb = post.tile([P, 2, L], f32r)          # normalized z^T
        wo_sb = post.tile([P, 2, D], f32r)
        wo_ap = wo_d.ap().rearrange("(dc p) o -> p dc o", p=P)
        nc.sync.dma_start(wo_sb[:, :, :], wo_ap[:, :, :])

        # attention, head pairs (even head on psum out partitions via v cols;
        # q/k chunks pair heads on partition halves for PE row-group overlap)
        def attn_block(hp, l0, w, interleave=None, tail=False):
            pair = (2 * hp, 2 * hp + 1)
            nq = w // 512
            zts = {
                (h, q2): psum.tile([P, 512], f32, tag="z", bufs=4,
                                   name=f"z_{h}_{q2}_{hp}_{l0}")
                for h in pair for q2 in range(nq)
            }

            def do_av(mc, h, E):
                for q2 in range(nq):
                    nc.tensor.matmul(
                        zts[(h, q2)][:],
                        v_sb[:, mc, h, :],
                        E[:, q2 * 512:(q2 + 1) * 512],
                        start=(mc == 0), stop=(mc == 15),
                    )

            # software-pipelined: AV of an older chunk issues after the S
            # matmuls of chunk mc, so each exp (ACT for even heads, DVE
            # custom op for odd) gets ~1.7us of slack off the critical path.
            # Full blocks lag 1 chunk; half-width blocks lag 2 (iterations
            # are half as long) and pack both heads' scores into one psum
            # tile so two slots cover the deeper pipeline.
            lag = 3 - nq
            pend = []
            for mc in range(16):
                curr = []
                S2 = (psum.tile([P, 1024], f32, tag="S", name=f"S2_{mc}")
                      if nq == 1 else None)
                for hi, h in enumerate(pair):
                    r0 = (h % 2) * 64
                    if nq == 1:
                        S = S2[:, hi * 512:(hi + 1) * 512]
                    else:
                        S_t = psum.tile([P, w], f32, tag="S", name=f"S_{mc}_{h}")
                        S = S_t[:]
                    for q2 in range(nq):
                        nc.tensor.matmul(
                            S[:, q2 * 512:(q2 + 1) * 512],
                            qk_sb[r0:r0 + 64, 2 + h // 2, mc * P:(mc + 1) * P],
                            qk_sb[r0:r0 + 64, h // 2,
                                  l0 + q2 * 512: l0 + (q2 + 1) * 512],
                            start=True, stop=True,
                        )
                    E = att.tile([P, w], f32r, tag="E", bufs=14)
                    # engine choice is strictly per-head: the DVE exp carries
                    # a c0^32 global factor that only cancels in the softmax
                    # ratio if every chunk of a row uses the same engine
                    if h % 2 == 0:
                        nc.scalar.activation(E[:], S[:], Exp, scale=32.0)
                    else:
                        nc.vector._custom_dve(
                            exp_op, out=E[:], in0=S[:],
                            s0=0.0, s1=_EXP_A, imm2=_EXP_B,
                        )
                    curr.append((mc, h, E))
                pend.append(curr)
                if len(pend) > lag:
                    for args in pend.pop(0):
                        do_av(*args)
                if interleave is not None and mc in interleave:
                    interleave[mc]()
            if tail:
                # final block: drain AVs head-major and normalize each head
                # entirely on the DVE the moment its accumulator stops, so
                # the tail out_proj unblocks as early as possible
                for hi, h in enumerate(pair):
                    for chunk in pend:
                        do_av(*chunk[hi])
                    zt = zts[(h, 0)]
                    rz = (h % 2) * 64
                    rsb = att.tile([P, 512], f32, tag="r", bufs=2,
                                   name=f"rt_{h}")
                    nc.vector.reciprocal_approx_fast(out=rsb[0:64, :],
                                                     in_=zt[0:64, :])
                    nc.vector.tensor_tensor(
                        zn_sb[rz:rz + 64, hp, l0:l0 + 512],
                        zt[64:128, :], rsb[0:64, :], Mult,
                    )
                return
            for chunk in pend:
                for args in chunk:
                    do_av(*args)
            # normalization split across three engines so the DVE (which also
            # carries half the exps) frees the z psum slots quickly: DVE only
            # computes reciprocals, ACT drains z out of psum, and the final
            # multiply runs on the otherwise-idle GPSIMD from SBUF.  qh-major
            # so the first token half unblocks consumers sooner
            for qh in range(nq):
                for h in pair:
                    zt = zts[(h, qh)]
                    rz = (h % 2) * 64
                    rsb = att.tile([P, 512], f32, tag="r", bufs=2)
                    zcp = att.tile([P, 512], f32, tag="zc", bufs=2)
                    nc.vector.reciprocal_approx_fast(out=rsb[0:64, :],
                                                     in_=zt[0:64, :])
                    nc.scalar.copy(zcp[0:64, :], zt[64:128, :])
                    nc.gpsimd.tensor_tensor(
                        zn_sb[rz:rz + 64, hp, l0 + qh * 512:l0 + (qh + 1) * 512],
                        zcp[0:64, :], rsb[0:64, :], Mult,
                    )

        # chunks the NEXT block reads go on the S tag (granted immediately
        # at the boundary); later-consumer prefetch rides the z tag and
        # overlaps the next block under the E-buffer slack.  (Producing the
        # gating chunks inside the previous block via S-tag inserts was
        # tried and measured worse — any theft from the S slot stream
        # stalls the exp pipeline more than the boundary gate costs.)
        def ilv(t, use_act=True):
            return lambda: out_proj(t, zn_sb, wo_sb, "S", use_act=use_act)

        # remaining projection chains are spread over the block boundaries
        # as PE filler sized to the DVE's normalization backlog; each chain
        # lands just before its consuming block (gating) or one block early
        # (prefetch)
        attn_block(0, 0, 1024)
        qk_proj(0, "S", fine=True, q4s=(2, 3), copy_eng="scalar")
        qk_proj(3, "z", fine=True, q4s=(0, 1, 2, 3))
        attn_block(0, 1024, 1024)
        qk_proj(1, "S", fine=True, q4s=(0, 1), copy_eng="scalar")
        qk_proj(1, "z", fine=True, q4s=(2, 3))
        attn_block(1, 0, 1024)
        # the last head-pair's q range runs as two 512-wide half-blocks so
        # out_proj tiles 8-11 interleave into the second half; only 12-15
        # remain for the serial tail
        attn_block(1, 1024, 512, interleave={
            mc: ilv((mc - 5) // 2, False) for mc in range(5, 16, 2)
        })
        attn_block(1, 1536, 512, interleave={
            3: ilv(6, False), 5: ilv(7, False), 7: ilv(8, False),
            9: ilv(9, False), 11: ilv(10, False), 13: ilv(11, False),
        }, tail=True)
        for t in range(12, 16):
            out_proj(t, zn_sb, wo_sb, "S")

    nc.compile()
    return nc


def _get_compiled():
    global _COMPILED
    if _COMPILED is None:
        _COMPILED = _build()
    return _COMPILED


def _shard_inputs(x, W_in, W_out):
    in_maps = []
    xTs = [x[:, b, :].T.astype(np.float16) for b in range(B)]
    for c in range(NCORES):
        b = c // 4
        lo = (c % 4) * J
        Wq = W_in[lo:lo + J]
        Wk = W_in[D + lo:D + lo + J]
        Wv = W_in[2 * D + lo:2 * D + lo + J]
        in_maps.append({
            "xT": xTs[b],
            # q weights carry the 1/256 pre-scale so scores arrive as S/256:
            # the ACT exp uses scale=32 (256/8) and the DVE exp op consumes
            # S/256 directly
            "wqkT": np.concatenate([Wq / 256.0, Wk], 0).T.astype(np.float16),
            "wvT": Wv.T.astype(np.float16),
            "woT": np.ascontiguousarray(W_out[:, lo:lo + J].T),
        })
    return in_maps


def _reference_numpy(q, mask, W_in, b_in, W_out, b_out, num_heads):
    l, b, d = q.shape
    hd = d // num_heads
    qkv = q.reshape(l * b, d) @ W_in.T + b_in
    qkv = qkv.reshape(l, b, 3 * d)
    qh, kh, vh = np.split(qkv, 3, axis=-1)

    def to_heads(t):
        return t.reshape(l, b * num_heads, hd).transpose(1, 0, 2)

    qh, kh, vh = to_heads(qh), to_heads(kh), to_heads(vh)
    qh = qh / np.sqrt(np.float32(hd))
    scores = np.einsum("nld,nmd->nlm", qh, kh) + mask
    scores -= scores.max(axis=-1, keepdims=True)
    e = np.exp(scores)
    attn = e / e.sum(axis=-1, keepdims=True)
    z = np.einsum("nlm,nmd->nld", attn, vh)
    z = z.transpose(1, 0, 2).reshape(l * b, d)
    z = z @ W_out.T + b_out
    return z.reshape(l, b, d).astype(np.float32)


def kernel(q, k, v, mask, W_in, b_in, W_out, b_out, num_heads):
    num_heads = int(num_heads)
    q = np.asarray(q, dtype=np.float32)
    W_in = np.asarray(W_in, dtype=np.float32)
    W_out = np.asarray(W_out, dtype=np.float32)
    b_in = np.asarray(b_in, dtype=np.float32)
    b_out = np.asarray(b_out, dtype=np.float32)
    mask = np.asarray(mask, dtype=np.float32)

    if (
        num_heads != H
        or q.shape != (L, B, D)
        or W_in.shape != (3 * D, D)
        or W_out.shape != (D, D)
        or np.any(mask)
        or np.any(b_in)
    ):
        return _reference_numpy(q, mask, W_in, b_in, W_out, b_out, num_heads)

    from concourse import bass_utils

    nc = _get_compiled()
    in_maps = _shard_inputs(q, W_in, W_out)
    res = bass_utils.run_bass_kernel_spmd(
        nc, in_maps, core_ids=list(range(NCORES))
    )

    out = np.zeros((L, B, D), dtype=np.float32)
    for c in range(NCORES):
        out[:, c // 4, :] += res.results[c]["out_p"].astype(np.float32)
    out += b_out
    return out



# revision 56
# speedup vs baseline: 1.0065x; 1.0010x over previous
"""Multi-head attention layer (L=2048, B=2, D=1024, H=16) on 8 Trainium2 cores.

Sharding: batch*heads across cores — core c handles batch c//4, heads
4*(c%4)..4*(c%4)+4.  Tensor-parallel W_in column slice (per-head) and W_out
row slice; per-core partial outputs are summed on the host (2 groups of 4).

Device program (identical SPMD program, per-core data).  The steady state is
PE-bound (~164us of matmul at f32r rate); everything else is arranged to
keep the tensor engine streaming:

  - inputs ride fp16 (xT (D,L), wqkT, wvT) / f32r (woT) in a few large DMAs
    (the HWDGE issue stage is a single 625ns/DMA resource) ordered so the
    projection waves chase the landings; dummy matmuls warm the PE p-state
    ramp during the initial DMA wait.
  - the softmax exp is split across two engines per head parity: even heads
    exp on ACT (exp(S*32)); odd heads run a custom 8-stage DVE op
    E = ((u+A)^2+B)^32 with u = S/256 (q weights pre-scaled 1/256 on the
    host).  The c0^32 scale cancels in the softmax ratio because each
    head's full rows stay on one engine.  This halves the former ACT
    bottleneck (~144us -> ~2x 100us in parallel).
  - attention blocks are software-pipelined: AV matmuls lag the S matmuls
    by one chunk (two for the 512-wide half-blocks, whose score tiles pack
    both heads into one psum tile) so exp latency is off the critical path.
  - v projection lands token-major with interleaved ones-columns so the AV
    matmul produces z^T and the softmax row sums in one pass; normalization
    is spread over DVE (reciprocal_approx), ACT (psum drain) and GPSIMD
    (multiply), keeping the DVE exp queue short at block boundaries.
  - remaining q/k projection chains sit at block boundaries as PE filler
    sized to the normalization drain; the final head-pair runs as two
    512-wide half-blocks so 12 of 16 output-projection tiles interleave
    into attention and only 4 remain in the serial tail; outputs ship as
    512KB paired DMAs (singles/halves at the very end to shorten the
    drain).
"""

import sys

for _p in ("/opt/trn_rl_repo",):
    if _p not in sys.path:
        sys.path.append(_p)

import numpy as np

L, B, D, H = 2048, 2, 1024, 16
HD = 64
NCORES = 8
HPC = 4              # heads per core
J = HPC * HD         # 256 per-core head-dim slice
KC = D // 128        # 8 contraction chunks
P = 128

_COMPILED = None

# DVE-offloaded exp: half the heads' softmax exponentials run on the Vector
# engine as a custom op E = ((u+A)^2+B)^32 with u = S/256 (q weights are
# pre-scaled 1/256 on the host so u arrives directly from the matmul).  The
# quadratic is a free-scale minimax fit of e^u on |u|<=0.2; the c0^32 global
# factor cancels in the softmax ratio because each head's full row uses one
# engine.  Rel err ~1e-2 at the +-6.4 sigma edges, ~4e-3 end-to-end per head.
_EXP_A = 1.0074990416831697
_EXP_B = 0.9900523184693841


def _register_exp_op():
    import concourse.dve_ops as dve_ops_mod
    from concourse.dve_ops import DveOp
    from concourse.dve_spec import Spec, Src0, C1, C2, sq, lower, _has_src1
    from concourse.dve_uop import DveOpSpec
    from concourse.dve_table_gen import dve_ver_for

    name = "EXP_MQ32_ANT"
    for op in dve_ops_mod.OPS:
        if op.name == name:
            return op

    p = sq(Src0 + C1) + C2
    for _ in range(5):
        p = sq(p)

    def ref(in0, in1, s0, s1, imm2):
        u = np.asarray(in0, np.float32)
        a = (u + np.float32(s1)).astype(np.float32)
        r = (a * a + np.float32(imm2)).astype(np.float32)
        for _ in range(5):
            r = (r * r).astype(np.float32)
        return r

    spec = Spec(body=p, reference=ref)
    row = max(dve_ops_mod._SUB_OPCODE_FOR_NAME.values()) + 1
    assert row < 0x20
    dve_ops_mod._SUB_OPCODE_FOR_NAME[name] = row
    ver = dve_ver_for("TRN2")
    sha = DveOpSpec(
        name=name, opcode=row, uops=lower(spec, ver=ver), rd1_en=_has_src1(spec)
    ).sha(ver)
    op = DveOp(name, spec, subdim=False, uops_sha={ver: sha})
    dve_ops_mod.OPS.append(op)
    dve_ops_mod.CUSTOM_DVE_SPECS[name] = spec
    return op


def _build():
    import concourse.bacc as bacc
    import concourse.mybir as mybir
    import concourse.tile as tile
    from contextlib import ExitStack

    exp_op = _register_exp_op()

    f32 = mybir.dt.float32
    f32r = mybir.dt.float32r
    f16 = mybir.dt.float16
    Exp = mybir.ActivationFunctionType.Exp
    Mult = mybir.AluOpType.mult

    nc = bacc.Bacc("TRN2", target_bir_lowering=False, debug=False)

    # x and the in-projection weights ride as fp16 (10-bit mantissa, on
    # par with f32r) to halve the prologue DMA; attention stays f32r
    xT_d = nc.dram_tensor("xT", (D, L), f16, kind="ExternalInput")
    wqk_d = nc.dram_tensor("wqkT", (D, 2 * J), f16, kind="ExternalInput")
    wv_d = nc.dram_tensor("wvT", (D, J), f16, kind="ExternalInput")
    wo_d = nc.dram_tensor("woT", (J, D), f32r, kind="ExternalInput")
    out_d = nc.dram_tensor("out_p", (L, D), f16, kind="ExternalOutput")

    with tile.TileContext(nc) as tc, ExitStack() as ctx:
        pers = ctx.enter_context(tc.tile_pool(name="pers", bufs=1))
        psum = ctx.enter_context(tc.tile_pool(name="psum", bufs=2, space="PSUM"))
        # z-tag tiles are [P,512] halves; 4 slots cover 2 heads x 2 halves
        att = ctx.enter_context(tc.tile_pool(name="att", bufs=3))

        qk_sb = pers.tile([P, 4, L], f32r)          # chunks 0,1: q^T; 2,3: k^T
        v_sb = pers.tile([P, 16, HPC, P], f32r)     # ones cols 0:64, v 64:128

        proj_a = ctx.enter_context(tc.tile_pool(name="proj_a", bufs=1))
        xT_sb = proj_a.tile([P, KC, L], f16)
        wqk_sb = proj_a.tile([P, KC, 2 * J], f16)

        out_ap = out_d.ap().rearrange("(t p) o -> p t o", p=P)

        def qk_proj(jc, tag, fine=False, q4s=(0, 1, 2, 3), copy_eng=None):
            if fine:
                # 512-wide psum chunks: shorter slot holds when slotted
                # between attention blocks on the shared z tag; copy_eng
                # picks which engine drains psum (boundary "gating" chunks
                # ride ACT so they don't queue behind normalization on DVE)
                for q4 in q4s:
                    m0 = q4 * 512
                    pt = psum.tile([P, 512], f32, tag=tag,
                                   bufs=(4 if tag == "z" else 2),
                                   name=f"qkp_{jc}_{q4}")
                    for kc in range(KC):
                        nc.tensor.matmul(
                            pt[:],
                            wqk_sb[:, kc, jc * P:(jc + 1) * P],
                            xT_sb[:, kc, m0:m0 + 512],
                            start=(kc == 0), stop=(kc == KC - 1),
                        )
                    if copy_eng == "scalar":
                        nc.scalar.copy(qk_sb[:, jc, m0:m0 + 512], pt[:])
                    else:
                        nc.vector.tensor_copy(
                            qk_sb[:, jc, m0:m0 + 512], pt[:]
                        )
                return
            for mh in range(2):
                pt = psum.tile([P, 1024], f32, tag=tag, name=f"qkp_{jc}_{mh}")
                for q2 in range(2):
                    m0 = mh * 1024 + q2 * 512
                    for kc in range(KC):
                        nc.tensor.matmul(
                            pt[:, q2 * 512:(q2 + 1) * 512],
                            wqk_sb[:, kc, jc * P:(jc + 1) * P],
                            xT_sb[:, kc, m0:m0 + 512],
                            start=(kc == 0), stop=(kc == KC - 1),
                        )
                nc.vector.tensor_copy(
                    qk_sb[:, jc, mh * 1024:(mh + 1) * 1024], pt[:]
                )

        # output tiles are buffered in pairs so each DMA moves 512KB — the
        # HWDGE issue stage (625ns, globally serialized) is the tail pacer
        out_pair = {}

        tail_pre = {}

        def out_proj(t, zn_sb, wo_sb, tag, use_act=False):
            if t in tail_pre:
                pos = tail_pre.pop(t)
                dcs = (1,)
            else:
                pos = [psum.tile([P, 512], f32, tag="z", bufs=4,
                                  name=f"po_{t}_{oc}")
                       for oc in range(2)]
                dcs = (0, 1)
            for oc in range(2):
                for dc in dcs:
                    nc.tensor.matmul(
                        pos[oc][:],
                        zn_sb[:, dc, t * P:(t + 1) * P],
                        wo_sb[:, dc, oc * 512:(oc + 1) * 512],
                        start=(dc == 0), stop=(dc == 1),
                    )
            if t % 2 == 0:
                out_pair[0] = att.tile([P, 2, 1024], f16, tag="o", bufs=2,
                                       name=f"op_{t}")
            pair = out_pair[0]
            nc.scalar.copy(pair[:, t % 2, 0:512], pos[0][:])
            if use_act:
                nc.scalar.copy(pair[:, t % 2, 512:1024], pos[1][:])
            else:
                nc.vector.tensor_copy(pair[:, t % 2, 512:1024], pos[1][:])
            if t == 15:
                # final halves issue from the (idle) ACT/DVE queues so the
                # SP sequencer's ~700ns-per-issue serialization of the t12-14
                # DMAs doesn't delay the very last transfer
                nc.scalar.dma_start(out_ap[:, 15, 0:512], pair[:, 1, 0:512])
                nc.sync.dma_start(out_ap[:, 15, 512:1024],
                                  pair[:, 1, 512:1024])
            elif t == 14:
                nc.scalar.dma_start(out_ap[:, 14, :], pair[:, 0])  # ACT queue
            elif t >= 12:
                nc.sync.dma_start(out_ap[:, t, :], pair[:, t % 2])
            elif t % 2 == 1:
                nc.sync.dma_start(out_ap[:, t - 1:t + 1, :], pair[:])

        with tc.tile_pool(name="proj_b", bufs=1) as proj_b:
            wv_sb = proj_b.tile([P, KC, J], f16)

            xT_ap = xT_d.ap().rearrange("(kc p) m -> p kc m", p=P)
            wqk_ap = wqk_d.ap().rearrange("(kc p) j -> p kc j", p=P)
            wv_ap = wv_d.ap().rearrange("(kc p) j -> p kc j", p=P)
            # few, large DMAs (the HWDGE issue stage is a single 625ns/DMA
            # resource), ordered so the projection waves below chase the
            # landings: q-weights for the first chain, then xT token-column
            # groups in consumption order
            nc.sync.dma_start(wqk_sb[:, :, 0:128], wqk_ap[:, :, 0:128])
            nc.sync.dma_start(xT_sb[:, 0:4, 0:512], xT_ap[:, 0:4, 0:512])
            nc.sync.dma_start(xT_sb[:, 4:8, 0:512], xT_ap[:, 4:8, 0:512])
            nc.sync.dma_start(wv_sb[:, :, :], wv_ap[:, :, :])
            nc.sync.dma_start(wqk_sb[:, :, 128:512], wqk_ap[:, :, 128:512])
            for cg in range(1, 4):
                nc.sync.dma_start(
                    xT_sb[:, :, cg * 512:(cg + 1) * 512],
                    xT_ap[:, :, cg * 512:(cg + 1) * 512],
                )

            # PE p-state warmup: the tensor engine needs ~3us of continuous
            # execution to reach full clock, so burn the initial DMA wait on
            # dummy matmuls and the projection chains start at full speed
            wz = proj_b.tile([P, 128], f32)
            nc.vector.memset(wz[:], 0.0)
            warm = wz[:].bitcast(f32r)
            for i in range(20):
                wp = psum.tile([P, 128], f32, tag="S", name=f"warm_{i}")
                nc.tensor.matmul(wp[:], warm[:], warm[:],
                                 start=True, stop=True)

            # ones columns at 0:64 for every head — keeps the softmax sums on
            # psum partitions 0-63 where the custom-DVE reciprocal works (it
            # silently corrupts at base partition 64).  memset on an f32r
            # tile fails the ISA check, so round through a f32 scratch tile.
            ones_sc = proj_b.tile([P, 64], f32)
            nc.vector.memset(ones_sc[:], 1.0)
            for h in range(HPC):
                nc.vector.tensor_copy(
                    v_sb[:, :, h, 0:64],
                    ones_sc[:, None, :].to_broadcast((P, 16, 64)),
                )

            # block (0,0) reads q-pair-0 for l 0:1024 but k-pair-0 for the
            # FULL key range; everything else is produced at the block
            # boundary just before its consumer.  Work is emitted in token-
            # column waves matching the xT DMA landing order; v_sb copies ride
            # the otherwise-idle scalar engine
            def v_proj(mg):
                pts = [psum.tile([P, 512], f32, tag="z", bufs=4,
                                 name=f"vp_{mg}_{i}")
                       for i in range(2)]
                for sub in range(4):
                    mc = mg * 4 + sub
                    pt = pts[sub // 2][:, (sub % 2) * 256:(sub % 2 + 1) * 256]
                    for kc in range(KC):
                        nc.tensor.matmul(
                            pt,
                            xT_sb[:, kc, mc * P:(mc + 1) * P],
                            wv_sb[:, kc, :],
                            start=(kc == 0), stop=(kc == KC - 1),
                        )
                for sub in range(4):
                    mc = mg * 4 + sub
                    dst = v_sb[:, mc, :, 64:128]
                    srcp = pts[sub // 2][:, (sub % 2) * 256:(sub % 2 + 1) * 256
                                         ].rearrange("p (h e) -> p h e", e=64)
                    if mg == 3:
                        nc.vector.tensor_copy(dst, srcp)
                    else:
                        nc.scalar.copy(dst, srcp)

            qk_proj(0, "S", fine=True, q4s=(0,))
            v_proj(0)
            qk_proj(2, "S", fine=True, q4s=(0,))
            qk_proj(0, "S", fine=True, q4s=(1,))
            qk_proj(2, "S", fine=True, q4s=(1,))
            v_proj(1)
            qk_proj(2, "S", fine=True, q4s=(2,))
            v_proj(2)
            qk_proj(2, "S", fine=True, q4s=(3,))
            v_proj(3)

        post = ctx.enter_context(tc.tile_pool(name="post", bufs=1))
        zn_sb = post.tile([P, 2, L], f32r)          # normalized z^T
        wo_sb = post.tile([P, 2, D], f32r)
        wo_ap = wo_d.ap().rearrange("(dc p) o -> p dc o", p=P)
        nc.sync.dma_start(wo_sb[:, :, :], wo_ap[:, :, :])

        # attention, head pairs (even head on psum out partitions via v cols;
        # q/k chunks pair heads on partition halves for PE row-group overlap)
        def attn_block(hp, l0, w, interleave=None, tail=False):
            pair = (2 * hp, 2 * hp + 1)
            nq = w // 512
            zts = {
                (h, q2): psum.tile([P, 512], f32, tag="z", bufs=4,
                                   name=f"z_{h}_{q2}_{hp}_{l0}")
                for h in pair for q2 in range(nq)
            }

            def do_av(mc, h, E):
                for q2 in range(nq):
                    nc.tensor.matmul(
                        zts[(h, q2)][:],
                        v_sb[:, mc, h, :],
                        E[:, q2 * 512:(q2 + 1) * 512],
                        start=(mc == 0), stop=(mc == 15),
                    )

            # software-pipelined: AV of an older chunk issues after the S
            # matmuls of chunk mc, so each exp (ACT for even heads, DVE
            # custom op for odd) gets ~1.7us of slack off the critical path.
            # Full blocks lag 1 chunk; half-width blocks lag 2 (iterations
            # are half as long) and pack both heads' scores into one psum
            # tile so two slots cover the deeper pipeline.
            lag = 3 - nq
            pend = []
            for mc in range(16):
                curr = []
                S2 = (psum.tile([P, 1024], f32, tag="S", name=f"S2_{mc}")
                      if nq == 1 else None)
                for hi, h in enumerate(pair):
                    r0 = (h % 2) * 64
                    if nq == 1:
                        S = S2[:, hi * 512:(hi + 1) * 512]
                    else:
                        S_t = psum.tile([P, w], f32, tag="S", name=f"S_{mc}_{h}")
                        S = S_t[:]
                    for q2 in range(nq):
                        nc.tensor.matmul(
                            S[:, q2 * 512:(q2 + 1) * 512],
                            qk_sb[r0:r0 + 64, 2 + h // 2, mc * P:(mc + 1) * P],
                            qk_sb[r0:r0 + 64, h // 2,
                                  l0 + q2 * 512: l0 + (q2 + 1) * 512],
                            start=True, stop=True,
                        )
                    E = att.tile([P, w], f32r, tag="E", bufs=14)
                    # engine choice is strictly per-head: the DVE exp carries
                    # a c0^32 global factor that only cancels in the softmax
                    # ratio if every chunk of a row uses the same engine
                    if h % 2 == 0:
                        nc.scalar.activation(E[:], S[:], Exp, scale=32.0)
                    else:
                        nc.vector._custom_dve(
                            exp_op, out=E[:], in0=S[:],
                            s0=0.0, s1=_EXP_A, imm2=_EXP_B,
                        )
                    curr.append((mc, h, E))
                pend.append(curr)
                if len(pend) > lag:
                    for args in pend.pop(0):
                        do_av(*args)
                if interleave is not None and mc in interleave:
                    interleave[mc]()
            if tail:
                # final block: drain AVs head-major and normalize each head
                # entirely on the DVE the moment its accumulator stops, so
                # the tail out_proj unblocks as early as possible
                for hi, h in enumerate(pair):
                    for chunk in pend:
                        do_av(*chunk[hi])
                    zt = zts[(h, 0)]
                    rz = (h % 2) * 64
                    rsb = att.tile([P, 512], f32, tag="r", bufs=2,
                                   name=f"rt_{h}")
                    nc.vector.reciprocal_approx_fast(out=rsb[0:64, :],
                                                     in_=zt[0:64, :])
                    nc.vector.tensor_tensor(
                        zn_sb[rz:rz + 64, hp, l0:l0 + 512],
                        zt[64:128, :], rsb[0:64, :], Mult,
                    )
                    if hi == 0:
                        # head-pair-0 halves of the first two tail tiles run
                        # on the now-idle S psum slots under the norm chain
                        for t in (12, 13):
                            tail_pre[t] = [
                                psum.tile([P, 512], f32, tag="S",
                                          name=f"tp_{t}_{oc}")
                                for oc in range(2)
                            ]
                            for oc in range(2):
                                nc.tensor.matmul(
                                    tail_pre[t][oc][:],
                                    zn_sb[:, 0, t * P:(t + 1) * P],
                                    wo_sb[:, 0, oc * 512:(oc + 1) * 512],
                                    start=True, stop=False,
                                )
                return
            for chunk in pend:
                for args in chunk:
                    do_av(*args)
            # normalization split across three engines so the DVE (which also
            # carries half the exps) frees the z psum slots quickly: DVE only
            # computes reciprocals, ACT drains z out of psum, and the final
            # multiply runs on the otherwise-idle GPSIMD from SBUF.  qh-major
            # so the first token half unblocks consumers sooner
            for qh in range(nq):
                for h in pair:
                    zt = zts[(h, qh)]
                    rz = (h % 2) * 64
                    rsb = att.tile([P, 512], f32, tag="r", bufs=2)
                    zcp = att.tile([P, 512], f32, tag="zc", bufs=2)
                    nc.vector.reciprocal_approx_fast(out=rsb[0:64, :],
                                                     in_=zt[0:64, :])
                    nc.scalar.copy(zcp[0:64, :], zt[64:128, :])
                    nc.gpsimd.tensor_tensor(
                        zn_sb[rz:rz + 64, hp, l0 + qh * 512:l0 + (qh + 1) * 512],
                        zcp[0:64, :], rsb[0:64, :], Mult,
                    )

        # chunks the NEXT block reads go on the S tag (granted immediately
        # at the boundary); later-consumer prefetch rides the z tag and
        # overlaps the next block under the E-buffer slack.  (Producing the
        # gating chunks inside the previous block via S-tag inserts was
        # tried and measured worse — any theft from the S slot stream
        # stalls the exp pipeline more than the boundary gate costs.)
        def ilv(t, use_act=True):
            return lambda: out_proj(t, zn_sb, wo_sb, "S", use_act=use_act)

        # remaining projection chains are spread over the block boundaries
        # as PE filler sized to the DVE's normalization backlog; each chain
        # lands just before its consuming block (gating) or one block early
        # (prefetch)
        attn_block(0, 0, 1024)
        qk_proj(0, "S", fine=True, q4s=(2, 3), copy_eng="scalar")
        qk_proj(3, "z", fine=True, q4s=(0, 1, 2, 3))
        attn_block(0, 1024, 1024)
        qk_proj(1, "S", fine=True, q4s=(0, 1), copy_eng="scalar")
        qk_proj(1, "z", fine=True, q4s=(2, 3))
        attn_block(1, 0, 1024)
        # the last head-pair's q range runs as two 512-wide half-blocks so
        # out_proj tiles 8-11 interleave into the second half; only 12-15
        # remain for the serial tail
        attn_block(1, 1024, 512, interleave={
            mc: ilv((mc - 5) // 2, False) for mc in range(5, 16, 2)
        })
        attn_block(1, 1536, 512, interleave={
            3: ilv(6, False), 5: ilv(7, False), 7: ilv(8, False),
            9: ilv(9, False), 11: ilv(10, False), 13: ilv(11, False),
        }, tail=True)
        for t in range(12, 16):
            out_proj(t, zn_sb, wo_sb, "S")

    nc.compile()
    return nc


def _get_compiled():
    global _COMPILED
    if _COMPILED is None:
        _COMPILED = _build()
    return _COMPILED


def _shard_inputs(x, W_in, W_out):
    in_maps = []
    xTs = [x[:, b, :].T.astype(np.float16) for b in range(B)]
    for c in range(NCORES):
        b = c // 4
        lo = (c % 4) * J
        Wq = W_in[lo:lo + J]
        Wk = W_in[D + lo:D + lo + J]
        Wv = W_in[2 * D + lo:2 * D + lo + J]
        in_maps.append({
            "xT": xTs[b],
            # q weights carry the 1/256 pre-scale so scores arrive as S/256:
            # the ACT exp uses scale=32 (256/8) and the DVE exp op consumes
            # S/256 directly
            "wqkT": np.concatenate([Wq / 256.0, Wk], 0).T.astype(np.float16),
            "wvT": Wv.T.astype(np.float16),
            "woT": np.ascontiguousarray(W_out[:, lo:lo + J].T),
        })
    return in_maps


def _reference_numpy(q, mask, W_in, b_in, W_out, b_out, num_heads):
    l, b, d = q.shape
    hd = d // num_heads
    qkv = q.reshape(l * b, d) @ W_in.T + b_in
    qkv = qkv.reshape(l, b, 3 * d)
    qh, kh, vh = np.split(qkv, 3, axis=-1)

    def to_heads(t):
        return t.reshape(l, b * num_heads, hd).transpose(1, 0, 2)

    qh, kh, vh = to_heads(qh), to_heads(kh), to_heads(vh)
    qh = qh / np.sqrt(np.float32(hd))
    scores = np.einsum("nld,nmd->nlm", qh, kh) + mask
    scores -= scores.max(axis=-1, keepdims=True)
    e = np.exp(scores)
    attn = e / e.sum(axis=-1, keepdims=True)
    z = np.einsum("nlm,nmd->nld", attn, vh)
    z = z.transpose(1, 0, 2).reshape(l * b, d)
    z = z @ W_out.T + b_out
    return z.reshape(l, b, d).astype(np.float32)


def kernel(q, k, v, mask, W_in, b_in, W_out, b_out, num_heads):
    num_heads = int(num_heads)
    q = np.asarray(q, dtype=np.float32)
    W_in = np.asarray(W_in, dtype=np.float32)
    W_out = np.asarray(W_out, dtype=np.float32)
    b_in = np.asarray(b_in, dtype=np.float32)
    b_out = np.asarray(b_out, dtype=np.float32)
    mask = np.asarray(mask, dtype=np.float32)

    if (
        num_heads != H
        or q.shape != (L, B, D)
        or W_in.shape != (3 * D, D)
        or W_out.shape != (D, D)
        or np.any(mask)
        or np.any(b_in)
    ):
        return _reference_numpy(q, mask, W_in, b_in, W_out, b_out, num_heads)

    from concourse import bass_utils

    nc = _get_compiled()
    in_maps = _shard_inputs(q, W_in, W_out)
    res = bass_utils.run_bass_kernel_spmd(
        nc, in_maps, core_ids=list(range(NCORES))
    )

    out = np.zeros((L, B, D), dtype=np.float32)
    for c in range(NCORES):
        out[:, c // 4, :] += res.results[c]["out_p"].astype(np.float32)
    out += b_out
    return out

